# revision 6
# baseline (speedup 1.0000x reference)
"""BiMatchLoss kernel for Trainium2 (8 NeuronCores, SPMD data-parallel over batch).

Math (validated vs reference):
  BCE(p,t) = -log1mp(p) - t*(logp(p) - log1mp(p))
  Summed over a bijective matching perm, the -log1mp part is perm-independent.
  So per batch b we only need (all computable in one pass over the data):
    cost[t,o]  = -sum_{s,ci} tgt[s,t,ci] * out[s,o,ci]          (argmin input)
    Gp[t,o]    =  sum_{s,ci} m[s]*tgt[s,t,ci] * logp[s,o,ci]
    Gm[t,o]    =  sum_{s,ci} m[s]*tgt[s,t,ci] * log1mp[s,o,ci]
    Amask      =  sum_{s,o,ci} m[s] * (-log1mp[s,o,ci])
  final = sum_b 0.5*(Amask_b - sum_t (Gp-Gm)[t, perm_b[t]]) / sum(m)

Device: matmuls over s (K=128 per tile, PSUM-accumulated over 8 tiles) with
bf16 operands; the (t,ci)x(o,ci') outer products are reduced to (t,o) via a
block-diagonal mask + grouped reduction. Host does the tiny 720-permutation
argmin and final assembly.
"""

import os
from itertools import permutations

import numpy as np
import ml_dtypes

import concourse.bacc as bacc
import concourse.mybir as mybir
from concourse.tile import TileContext
from concourse.bass_utils import run_bass_kernel_spmd

B, S, E, C = 32, 1024, 6, 16
F = E * C * 2          # 192 flattened (e, c, i)
CI = C * 2             # 32
NCORE = 8
NB = B // NCORE        # 4 batches per core
NT = S // 128          # 8 s-tiles per batch

f32 = mybir.dt.float32
bf16 = mybir.dt.bfloat16
AF = mybir.ActivationFunctionType
ALU = mybir.AluOpType

_PROG = None           # cached compiled Bass program
LAST = None            # last BassKernelResults (for test.py timing)


def _build_program():
    nc = bacc.Bacc("TRN2", target_bir_lowering=False, debug=False,
                   num_devices=1)

    xo_d = nc.dram_tensor("xo", [NB, S, F], bf16, kind="ExternalInput").ap()
    xt_d = nc.dram_tensor("xt", [NB, S, F], bf16, kind="ExternalInput").ap()
    mcol_d = nc.dram_tensor("mcol", [128, NB * NT], f32,
                            kind="ExternalInput").ap()
    dmask_d = nc.dram_tensor("dmask", [128, 2 * F], bf16,
                             kind="ExternalInput").ap()
    red_d = nc.dram_tensor("red", [NB, 128, 36], f32,
                           kind="ExternalOutput").ap()
    amask_d = nc.dram_tensor("amask", [NB, 128], f32,
                             kind="ExternalOutput").ap()

    with TileContext(nc) as tc:
        with (
            tc.tile_pool(name="consts", bufs=1) as cpool,
            tc.tile_pool(name="io", bufs=4) as iop,
            tc.tile_pool(name="mid", bufs=3) as midp,
            tc.tile_pool(name="post", bufs=2) as postp,
            tc.tile_pool(name="ps", bufs=2, space="PSUM") as psp,
        ):
            mcol_sb = cpool.tile([128, NB * NT], f32)
            nc.sync.dma_start(mcol_sb[:], mcol_d)
            dmask_sb = cpool.tile([128, 2 * F], bf16)
            nc.sync.dma_start(dmask_sb[:], dmask_d)

            for b in range(NB):
                pc1 = psp.tile([128, F], f32, tag="pc1")
                pc2 = psp.tile([64, F], f32, tag="pc2")
                pg1 = psp.tile([128, 2 * F], f32, tag="pg1")
                pg2 = psp.tile([64, 2 * F], f32, tag="pg2")
                arow = midp.tile([128, NT], f32, tag="arow")

                for k in range(NT):
                    xo_t = iop.tile([128, F], bf16, tag="xo_t")
                    nc.sync.dma_start(xo_t[:], xo_d[b, k * 128:(k + 1) * 128, :])
                    xt_t = iop.tile([128, F], bf16, tag="xt_t")
                    nc.sync.dma_start(xt_t[:], xt_d[b, k * 128:(k + 1) * 128, :])

                    # logs: cols 0:F = log(p), F:2F = log(1-p)
                    logs = midp.tile([128, 2 * F], bf16, tag="logs")
                    nc.scalar.activation(logs[:, 0:F], xo_t[:], AF.Ln)
                    nc.scalar.activation(logs[:, F:2 * F], xo_t[:], AF.Ln,
                                         bias=1.0, scale=-1.0,
                                         accum_out=arow[:, k:k + 1])

                    # masked targets (bf16): tgtm = tgt * m[s]
                    tgtm = midp.tile([128, F], bf16, tag="tgtm")
                    nc.vector.tensor_scalar(
                        tgtm[:], xt_t[:],
                        mcol_sb[:, b * NT + k:b * NT + k + 1], None, ALU.mult)

                    st = dict(start=(k == 0), stop=(k == NT - 1))
                    nc.tensor.matmul(pc1[:], xt_t[:, 0:128], xo_t[:], **st)
                    nc.tensor.matmul(pc2[:], xt_t[:, 128:F], xo_t[:], **st)
                    nc.tensor.matmul(pg1[:], tgtm[:, 0:128], logs[:], **st)
                    nc.tensor.matmul(pg2[:], tgtm[:, 128:F], logs[:], **st)

                # block-diag extraction: tmp = psum * dmask, then grouped sums
                tmp_c = postp.tile([128, F], f32, tag="tmp_c")
                nc.vector.tensor_tensor(tmp_c[:], pc1[:], dmask_sb[:, 0:F],
                                        ALU.mult)
                tmp_c2 = postp.tile([64, F], f32, tag="tmp_c2")
                nc.vector.tensor_tensor(tmp_c2[:], pc2[:], dmask_sb[0:64, 0:F],
                                        ALU.mult)
                tmp_g = postp.tile([128, 2 * F], f32, tag="tmp_g")
                nc.vector.tensor_tensor(tmp_g[:], pg1[:], dmask_sb[:], ALU.mult)
                tmp_g2 = postp.tile([64, 2 * F], f32, tag="tmp_g2")
                nc.vector.tensor_tensor(tmp_g2[:], pg2[:], dmask_sb[0:64, :],
                                        ALU.mult)

                red_sb = postp.tile([128, 36], f32, tag="red_sb")
                nc.gpsimd.memset(red_sb[:], 0.0)
                nc.vector.tensor_reduce(
                    red_sb[:, 0:6], tmp_c[:].rearrange("p (o j) -> p o j", j=CI),
                    mybir.AxisListType.X, ALU.add)
                nc.vector.tensor_reduce(
                    red_sb[0:64, 6:12],
                    tmp_c2[:].rearrange("p (o j) -> p o j", j=CI),
                    mybir.AxisListType.X, ALU.add)
                nc.vector.tensor_reduce(
                    red_sb[:, 12:24],
                    tmp_g[:].rearrange("p (o j) -> p o j", j=CI),
                    mybir.AxisListType.X, ALU.add)
                nc.vector.tensor_reduce(
                    red_sb[0:64, 24:36],
                    tmp_g2[:].rearrange("p (o j) -> p o j", j=CI),
                    mybir.AxisListType.X, ALU.add)
                nc.sync.dma_start(red_d[b], red_sb[:])

                # Amask partials: sum_k m[:,k]*arow[:,k] per partition
                junk = postp.tile([128, NT], f32, tag="junk")
                am_col = postp.tile([128, 1], f32, tag="am_col")
                nc.vector.tensor_tensor(junk[:], arow[:],
                                        mcol_sb[:, b * NT:(b + 1) * NT],
                                        ALU.mult)
                nc.vector.tensor_reduce(am_col[:], junk[:],
                                        mybir.AxisListType.X, ALU.add)
                nc.sync.dma_start(amask_d[b, :], am_col[:])

    nc.compile()
    return nc


def _get_program():
    global _PROG
    if _PROG is None:
        _PROG = _build_program()
    return _PROG


def kernel(outputs, targets, attention_mask):
    global LAST
    out_np = np.asarray(outputs, dtype=np.float32)
    tgt_np = np.asarray(targets, dtype=np.float32)
    m_np = np.asarray(attention_mask)

    xo_all = out_np.reshape(B, S, F).astype(ml_dtypes.bfloat16)
    xt_all = tgt_np.reshape(B, S, F).astype(ml_dtypes.bfloat16)

    # dmask[p, q] = 1 where p%32 == q%32 (block-diagonal selector)
    p_idx = np.arange(128)[:, None] % CI
    q_idx = np.arange(2 * F)[None, :] % CI
    dmask = (p_idx == q_idx).astype(ml_dtypes.bfloat16)

    in_maps = []
    for c in range(NCORE):
        bs = slice(c * NB, (c + 1) * NB)
        m_core = m_np[bs].astype(np.float32)          # [NB, S]
        # mcol[p, b*NT+k] = m[b, k*128+p]
        mcol = np.ascontiguousarray(
            m_core.reshape(NB, NT, 128).transpose(2, 0, 1).reshape(128, NB * NT))
        in_maps.append({
            "xo": np.ascontiguousarray(xo_all[bs]),
            "xt": np.ascontiguousarray(xt_all[bs]),
            "mcol": mcol,
            "dmask": dmask,
        })

    nc = _get_program()
    res = run_bass_kernel_spmd(nc, in_maps, list(range(NCORE)))
    LAST = res

    P = np.array(list(permutations(range(E))), dtype=np.int32)
    t_idx = np.arange(E)[None, :]
    ar = np.arange(E)
    num = 0.0
    for c in range(NCORE):
        red = res.results[c]["red"]      # [NB, 128, 36] f32
        am = res.results[c]["amask"]     # [NB, 128] f32
        for b in range(NB):
            rb = red[b]

            def blocks(cols6_hi, cols6_lo):
                hi = rb[:, cols6_hi].reshape(4, 32, 6).sum(1, dtype=np.float32)
                lo = rb[0:64, cols6_lo].reshape(2, 32, 6).sum(1, dtype=np.float32)
                return np.concatenate([hi, lo], axis=0)      # [6,6] (t,o)

            cost = -blocks(slice(0, 6), slice(6, 12))
            Gp = blocks(slice(12, 18), slice(24, 30))
            Gm = blocks(slice(18, 24), slice(30, 36))
            G = Gp - Gm

            totals = cost[t_idx, P].sum(-1, dtype=np.float32)
            perm = P[int(np.argmin(totals))]
            amask_b = -am[b].sum(dtype=np.float64)
            num += 0.5 * (amask_b - float(G[ar, perm].sum(dtype=np.float64)))

    den = float(m_np.sum())
    return np.float32(num / den)
